# revision 1
# baseline (speedup 1.0000x reference)
"""Multi-head causal attention (B=4, S=2048, D=1024, H=16, Dh=64) on 8 TRN2 NeuronCores.

Sharding: core c = (b = c//2, head-half = c%2); each core computes 8 heads of one
batch. Outputs: per-head probs [8,2048,2048] (causal lower blocks only; output
buffers are pre-zeroed) and a partial out [2048,1024] (its 8 heads' ctx @ W_O).
Host sums the two partials per batch and adds b_O.
"""
import os
import numpy as np
from contextlib import ExitStack

import concourse.bass as bass
import concourse.tile as tile
from concourse import bacc, mybir
from concourse.bass_utils import run_bass_kernel_spmd

F32 = mybir.dt.float32
F32R = mybir.dt.float32r

B, S, D, H, DH = 4, 2048, 1024, 16, 64
NPAIR = 4          # head pairs per core
NQC = S // 128     # 16 q-chunks
NEG = -1e30
_nc_cache = {}


def _build():
    nc = bacc.Bacc()
    xT = nc.dram_tensor("xT", [D, S], F32R, kind="ExternalInput")
    wq = nc.dram_tensor("wq", [NPAIR, D, 128], F32R, kind="ExternalInput")
    wk = nc.dram_tensor("wk", [NPAIR, D, 128], F32R, kind="ExternalInput")
    wv = nc.dram_tensor("wv", [NPAIR, D, 128], F32R, kind="ExternalInput")
    wo = nc.dram_tensor("wo", [NPAIR, 128, D], F32R, kind="ExternalInput")
    bq = nc.dram_tensor("bq", [128, NPAIR], F32, kind="ExternalInput")
    bk = nc.dram_tensor("bk", [128, NPAIR], F32, kind="ExternalInput")
    bv = nc.dram_tensor("bv", [128, NPAIR], F32, kind="ExternalInput")
    maskd = nc.dram_tensor("maskd", [128, 128], F32, kind="ExternalInput")
    ident = nc.dram_tensor("ident", [128, 128], F32R, kind="ExternalInput")

    probs = nc.dram_tensor("probs", [2 * NPAIR, S, S], F32, kind="ExternalOutput")
    outp = nc.dram_tensor("outp", [S, D], F32, kind="ExternalOutput")

    xT_r = xT.rearrange("(dc dp) s -> dp dc s", dp=128)   # [128, 8, 2048]

    with tile.TileContext(nc) as tc, ExitStack() as ctx:
        consts = ctx.enter_context(tc.tile_pool(name="consts", bufs=1))
        weights = ctx.enter_context(tc.tile_pool(name="weights", bufs=1))
        xtp = ctx.enter_context(tc.tile_pool(name="xtp", bufs=2))
        big = ctx.enter_context(tc.tile_pool(name="big", bufs=1))
        prows = ctx.enter_context(tc.tile_pool(name="prows", bufs=3))
        erows = ctx.enter_context(tc.tile_pool(name="erows", bufs=2))
        pts = ctx.enter_context(tc.tile_pool(name="pts", bufs=4))
        outs = ctx.enter_context(tc.tile_pool(name="outs", bufs=3))
        small = ctx.enter_context(tc.tile_pool(name="small", bufs=4))
        ps_qkv = ctx.enter_context(tc.tile_pool(name="ps_qkv", bufs=2, space="PSUM"))
        ps_sc = ctx.enter_context(tc.tile_pool(name="ps_sc", bufs=2, space="PSUM"))
        ps_tr = ctx.enter_context(tc.tile_pool(name="ps_tr", bufs=2, space="PSUM"))
        ps_ctx = ctx.enter_context(tc.tile_pool(name="ps_ctx", bufs=2, space="PSUM"))

        sb_mask = consts.tile([128, 128], F32)
        nc.sync.dma_start(out=sb_mask[:], in_=maskd[:])
        sb_id = consts.tile([128, 128], F32R)
        nc.sync.dma_start(out=sb_id[:], in_=ident[:])
        sb_bq = consts.tile([128, NPAIR], F32)
        nc.sync.dma_start(out=sb_bq[:], in_=bq[:])
        sb_bk = consts.tile([128, NPAIR], F32)
        nc.sync.dma_start(out=sb_bk[:], in_=bk[:])
        sb_bv = consts.tile([128, NPAIR], F32)
        nc.sync.dma_start(out=sb_bv[:], in_=bv[:])
        wo_sb = consts.tile([128, NPAIR, D], F32R)
        nc.sync.dma_start(out=wo_sb[:], in_=wo[:].transpose([1, 0, 2]))

        ctx_all = big.tile([128, NPAIR, S], F32R)   # [e2, pair, q]

        for p in range(NPAIR):
            wq_sb = weights.tile([128, 8, 128], F32R, tag="wq")
            nc.sync.dma_start(out=wq_sb[:], in_=wq[p].rearrange("(dc dp) m -> dp dc m", dp=128))
            wk_sb = weights.tile([128, 8, 128], F32R, tag="wk")
            nc.sync.dma_start(out=wk_sb[:], in_=wk[p].rearrange("(dc dp) m -> dp dc m", dp=128))
            wv_sb = weights.tile([128, 8, 128], F32R, tag="wv")
            nc.sync.dma_start(out=wv_sb[:], in_=wv[p].rearrange("(dc dp) m -> dp dc m", dp=128))

            q2t = big.tile([128, S], F32R, tag="q2t")
            k2t = big.tile([128, S], F32R, tag="k2t")
            v2 = big.tile([128, NQC, 128], F32R, tag="v2")   # [s-part, kc, e2]

            # ---- QKV projections (transposed layout) ----
            for w in range(4):
                sw = slice(w * 512, (w + 1) * 512)
                xt_t = xtp.tile([128, 8, 512], F32R, tag="xt")
                nc.sync.dma_start(out=xt_t[:], in_=xT_r[:, :, sw])
                for wsb, dest, bias in ((wq_sb, q2t, sb_bq), (wk_sb, k2t, sb_bk)):
                    ps = ps_qkv.tile([128, 512], F32, tag="qkv")
                    for dc in range(8):
                        nc.tensor.matmul(ps[:], wsb[:, dc, :], xt_t[:, dc, :],
                                         start=(dc == 0), stop=(dc == 7))
                    nc.vector.tensor_scalar_add(dest[:, sw], ps[:], bias[:, p:p + 1])
                # V: project then transpose to [s, e2]
                ps = ps_qkv.tile([128, 512], F32, tag="qkv")
                for dc in range(8):
                    nc.tensor.matmul(ps[:], wv_sb[:, dc, :], xt_t[:, dc, :],
                                     start=(dc == 0), stop=(dc == 7))
                v2t_tmp = pts.tile([128, 512], F32R, tag="v2t")
                nc.vector.tensor_scalar_add(v2t_tmp[:], ps[:], sb_bv[:, p:p + 1])
                for c in range(4):
                    kc = w * 4 + c
                    ptr = ps_tr.tile([128, 128], F32R, tag="tr")
                    nc.tensor.transpose(ptr[:], v2t_tmp[:, c * 128:(c + 1) * 128], sb_id[:])
                    nc.vector.tensor_copy(v2[:, kc, :], ptr[:])

            # ---- attention ----
            for W in range(4):
                for h in range(2):
                    e0 = h * 64
                    hh = p * 2 + h
                    ctx_ps = ps_ctx.tile([64, 512], F32, tag="ctx")
                    for di in range(4):
                        i = 4 * W + di
                        ki = 128 * (i + 1)
                        nw = W + 1
                        erow = erows.tile([128, S], F32, tag="erow")
                        prow = prows.tile([128, S], F32R, tag="prow")
                        acc = small.tile([128, 4], F32, tag="acc")
                        for jw in range(nw):
                            wid = 512 if jw < nw - 1 else 128 * (di + 1)
                            j0 = jw * 512
                            ps = ps_sc.tile([128, 512], F32, tag="sc")
                            nc.tensor.matmul(ps[:, :wid],
                                             q2t[e0:e0 + 64, i * 128:(i + 1) * 128],
                                             k2t[e0:e0 + 64, j0:j0 + wid],
                                             start=True, stop=True)
                            if jw == nw - 1:
                                nc.vector.tensor_add(ps[:, wid - 128:wid],
                                                     ps[:, wid - 128:wid], sb_mask[:])
                            nc.scalar.activation(erow[:, j0:j0 + wid], ps[:, :wid],
                                                 mybir.ActivationFunctionType.Exp,
                                                 bias=0.0, scale=0.125,
                                                 accum_out=acc[:, jw:jw + 1])
                        rs = small.tile([128, 1], F32, tag="rs")
                        nc.vector.tensor_reduce(rs[:], acc[:, :nw],
                                                mybir.AxisListType.X, mybir.AluOpType.add)
                        rec = small.tile([128, 1], F32, tag="rec")
                        nc.vector.reciprocal(rec[:], rs[:])
                        nc.vector.tensor_scalar_mul(prow[:, :ki], erow[:, :ki], rec[:])
                        nc.sync.dma_start(out=probs[hh, i * 128:(i + 1) * 128, 0:ki],
                                          in_=prow[:, :ki].bitcast(F32))
                        for kc in range(i + 1):
                            ptr = ps_tr.tile([128, 128], F32R, tag="tr")
                            nc.tensor.transpose(ptr[:], prow[:, kc * 128:(kc + 1) * 128], sb_id[:])
                            pt = pts.tile([128, 128], F32R, tag="pt")
                            if kc % 2 == 0:
                                nc.vector.tensor_copy(pt[:], ptr[:])
                            else:
                                nc.scalar.copy(pt[:], ptr[:])
                            nc.tensor.matmul(ctx_ps[:, di * 128:(di + 1) * 128],
                                             v2[:, kc, e0:e0 + 64], pt[:],
                                             start=(kc == 0), stop=(kc == i))
                    nc.vector.tensor_copy(ctx_all[e0:e0 + 64, p, W * 512:(W + 1) * 512], ctx_ps[:])

        # ---- output projection: accumulate over pairs ----
        for qc in range(NQC):
            o = outs.tile([128, D], F32, tag="o")
            for dwin in range(2):
                ps = ps_qkv.tile([128, 512], F32, tag="qkv")
                for p in range(NPAIR):
                    nc.tensor.matmul(ps[:], ctx_all[:, p, qc * 128:(qc + 1) * 128],
                                     wo_sb[:, p, dwin * 512:(dwin + 1) * 512],
                                     start=(p == 0), stop=(p == NPAIR - 1))
                nc.scalar.copy(o[:, dwin * 512:(dwin + 1) * 512], ps[:])
            nc.sync.dma_start(out=outp[qc * 128:(qc + 1) * 128, :], in_=o[:])

    nc.compile()
    return nc


def _prep_inputs(x, W_Q, W_K, W_V, W_O, b_Q, b_K, b_V):
    """Build the 8 per-core input dicts."""
    mask = np.where(np.arange(128)[None, :] <= np.arange(128)[:, None],
                    np.float32(0.0), np.float32(NEG)).astype(np.float32)
    ident = np.eye(128, dtype=np.float32)
    xTs = [np.ascontiguousarray(x[b].T) for b in range(B)]

    def stack_w(Wm, h0):  # [H, D, DH] -> [NPAIR, D, 128]
        return np.ascontiguousarray(
            Wm[h0:h0 + 8].reshape(NPAIR, 2, D, DH).transpose(0, 2, 1, 3).reshape(NPAIR, D, 128))

    def stack_wo(Wm, h0):  # [H, DH, D] -> [NPAIR, 128, D]
        return np.ascontiguousarray(Wm[h0:h0 + 8].reshape(NPAIR, 128, D))

    def stack_b(bm, h0):  # [H, DH] -> [128, NPAIR]
        return np.ascontiguousarray(bm[h0:h0 + 8].reshape(NPAIR, 128).T)

    in_maps = []
    for c in range(8):
        b, hq = c // 2, c % 2
        h0 = hq * 8
        in_maps.append({
            "xT": xTs[b],
            "wq": stack_w(W_Q, h0), "wk": stack_w(W_K, h0), "wv": stack_w(W_V, h0),
            "wo": stack_wo(W_O, h0),
            "bq": stack_b(b_Q, h0), "bk": stack_b(b_K, h0), "bv": stack_b(b_V, h0),
            "maskd": mask, "ident": ident,
        })
    return in_maps


def run(inputs, trace=False):
    """Returns ((probs, out), exec_time_ns_or_None)."""
    x = np.asarray(inputs["x"], dtype=np.float32)
    W_Q = np.asarray(inputs["W_Q"], dtype=np.float32)
    W_K = np.asarray(inputs["W_K"], dtype=np.float32)
    W_V = np.asarray(inputs["W_V"], dtype=np.float32)
    W_O = np.asarray(inputs["W_O"], dtype=np.float32)
    b_Q = np.asarray(inputs["b_Q"], dtype=np.float32)
    b_K = np.asarray(inputs["b_K"], dtype=np.float32)
    b_V = np.asarray(inputs["b_V"], dtype=np.float32)
    b_O = np.asarray(inputs["b_O"], dtype=np.float32)

    if "nc" not in _nc_cache:
        _nc_cache["nc"] = _build()
    nc = _nc_cache["nc"]

    in_maps = _prep_inputs(x, W_Q, W_K, W_V, W_O, b_Q, b_K, b_V)
    kw = {}
    if trace:
        kw = dict(trace=True, trace_cores=[0])
    res = run_bass_kernel_spmd(nc, in_maps, core_ids=list(range(8)), **kw)

    probs_full = np.empty((B, H, S, S), dtype=np.float32)
    out_full = np.empty((B, S, D), dtype=np.float32)
    for b in range(B):
        probs_full[b, 0:8] = res.results[2 * b]["probs"]
        probs_full[b, 8:16] = res.results[2 * b + 1]["probs"]
        out_full[b] = res.results[2 * b]["outp"] + res.results[2 * b + 1]["outp"] + b_O
    return (probs_full, out_full), res.exec_time_ns


def kernel(**inputs):
    (probs_full, out_full), _ = run(inputs, trace=False)
    return (probs_full, out_full)
